# revision 20
# baseline (speedup 1.0000x reference)
"""Trainium2 Bass kernel for nn_BlockAttnResRouter (moe_routing).

Reference computation (per token, over N=6 sources of width D=1024):
    normed   = rms_norm(sources) * norm_weight
    scores_n = dot(w[router_idx], normed_n)
    alphas   = softmax(scores + bias)         # bias on source 0 and N-1
    h        = sum_n alphas_n * sources_n     # raw sources, not normed
    alpha_mem = alphas[..., 0]

Contract: kernel(**inputs) takes the FULL unsharded numpy inputs and returns
(h, alpha_mem) like the reference. Inside, the B*S = 16384 tokens are split
evenly across 8 NeuronCores (data parallel); the tiny router params are
folded on the host (wv = w[idx] * norm_weight, bias vector) and replicated.

Per-core device kernel (Tile framework), per 128-token tile:
  - DMA one [128, 6*1024] f32 tile of sources (3.1 MB, contiguous lines)
  - ScalarE: Square activation with accum_out -> sum(x^2) per (token, n)
  - VectorE: affine_mul_reduce (custom DVE op) -> dot(wv, x) per (token, n)
  - rstd = exp(-0.5 * ln(msq + eps)) on ScalarE (one table set: ln+exp)
  - softmax over n: max via ttr, Exp with per-partition bias=-max and
    accum_out -> sum, reciprocal on VectorE
  - weighted sum via TensorE: for each n, matmul(diag(alpha_n), x_n)
    accumulated in PSUM (diag built as eye * e_n * r with tensor_scalar)
  - ScalarE copies PSUM -> SBUF, DMA h tile out; alpha_mem staged and
    written once at the end.
Token index = p*NT + t (partition-major) so all DMA APs have large
contiguous per-partition lines.
"""

import numpy as np

B, S, N, D = 4, 4096, 6, 1024
EPS = 1e-6
NCORES = 8
TOK = B * S          # 16384 tokens
TPC = TOK // NCORES  # 2048 tokens per core
P = 128              # SBUF partitions
DH = 512             # one fp32 PSUM bank of free dim

USE_F32R = False     # fp32r (4x faster PE) needs producers rounded to fp32r;
                     # plain fp32 matmul (4 cyc/row) is exact and ~= DMA budget

_nc_cache = {}


def build_nc(tpc=TPC, use_f32r=USE_F32R, x_bufs=3):
    """Build + compile the single-core Bass module for `tpc` tokens."""
    from contextlib import ExitStack

    import concourse.bacc as bacc
    import concourse.mybir as mybir
    import concourse.tile as tile
    from concourse.masks import make_identity

    f32 = mybir.dt.float32
    AF = mybir.ActivationFunctionType
    OP = mybir.AluOpType

    nt = tpc // P
    assert nt * P == tpc

    # Bacc (not plain Bass): its finalize() runs generate_event_semaphores
    # (walrus allows at most 1 sem wait per compute instruction) and
    # codegen_inst_isa_subclasses (encodes custom DVE ops like
    # affine_mul_reduce) -- without these, walrus codegen rejects the BIR.
    nc = bacc.Bacc()
    src = nc.declare_dram_parameter("src", [tpc, N * D], f32, isOutput=False)
    wv = nc.declare_dram_parameter("wv", [D], f32, isOutput=False)
    bias = nc.declare_dram_parameter("bias", [N], f32, isOutput=False)
    h = nc.declare_dram_parameter("h", [tpc, D], f32, isOutput=True)
    am = nc.declare_dram_parameter("am", [tpc], f32, isOutput=True)

    # token = p*nt + t: per-partition DRAM lines stay contiguous for all DMAs
    src_v = src[:].rearrange("(p t) (n d) -> t p n d", t=nt, n=N)
    h_v = h[:].rearrange("(p t) d -> t p d", t=nt)
    am_v = am[:].rearrange("(p t) -> p t", t=nt)

    with tile.TileContext(nc) as tc, ExitStack() as ctx:
        singles = ctx.enter_context(tc.tile_pool(name="singles", bufs=1))
        xp = ctx.enter_context(tc.tile_pool(name="x", bufs=x_bufs))
        hpool = ctx.enter_context(tc.tile_pool(name="h", bufs=3))
        dpool = ctx.enter_context(tc.tile_pool(name="dmat", bufs=3))
        sp = ctx.enter_context(tc.tile_pool(name="small", bufs=6))
        pp = ctx.enter_context(tc.tile_pool(name="psum", bufs=2, space="PSUM"))

        eye = singles.tile([P, P], f32)
        make_identity(nc, eye)
        wvb = singles.tile([P, D], f32)
        nc.gpsimd.dma_start(out=wvb, in_=wv[None, :].to_broadcast([P, D]))
        biasb = singles.tile([P, N], f32)
        nc.gpsimd.dma_start(out=biasb, in_=bias[None, :].to_broadcast([P, N]))
        am_stage = singles.tile([P, nt], f32)
        eps_tile = singles.tile([P, 1], f32)
        nc.vector.memset(eps_tile, EPS)
        # throwaway targets for ops whose only useful result is accum_out
        act_dummy = singles.tile([P, 1], f32)
        dve_dummy = singles.tile([P, 1], f32)

        for t in range(nt):
            x = xp.tile([P, N, D], f32)
            nc.sync.dma_start(out=x, in_=src_v[t])

            sumsq = sp.tile([P, N], f32)
            dots = sp.tile([P, N], f32)
            for n in range(N):
                nc.scalar.activation(
                    out=act_dummy.broadcast_to((P, D)),
                    in_=x[:, n, :],
                    func=AF.Square,
                    accum_out=sumsq[:, n : n + 1],
                )
                nc.vector.affine_mul_reduce(
                    out=dve_dummy.broadcast_to((P, D)),
                    accum_out=dots[:, n : n + 1],
                    in0=x[:, n, :],
                    in1=wvb,
                    scale=1.0,
                    bias=0.0,
                )

            # rstd = (mean(x^2) + eps) ** -0.5, via ln+exp (same ACT table set
            # as the softmax Exp; Rsqrt activation is banned for accuracy)
            lv = sp.tile([P, N], f32)
            nc.scalar.activation(
                out=lv, in_=sumsq, func=AF.Ln, scale=1.0 / D, bias=eps_tile
            )
            rstd = sp.tile([P, N], f32)
            nc.scalar.activation(out=rstd, in_=lv, func=AF.Exp, scale=-0.5)

            sc = sp.tile([P, N], f32)
            nc.vector.tensor_mul(sc, dots, rstd)
            sb = sp.tile([P, N], f32)
            m = sp.tile([P, 1], f32)
            nc.vector.tensor_add(sb, sc, biasb)
            nc.vector.reduce_max(m, sb, axis=mybir.AxisListType.X)
            negm = sp.tile([P, 1], f32)
            nc.vector.tensor_scalar_mul(negm, m, -1.0)
            es = sp.tile([P, N], f32)
            sume = sp.tile([P, 1], f32)
            nc.scalar.activation(
                out=es, in_=sb, func=AF.Exp, bias=negm, scale=1.0, accum_out=sume
            )
            r = sp.tile([P, 1], f32)
            nc.vector.reciprocal(r, sume)
            nc.vector.tensor_scalar_mul(am_stage[:, t : t + 1], es[:, 0:1], r)

            # diag(alpha_n) = eye * e_n * r, one tensor_scalar each
            dm = dpool.tile([P, N, P], f32)
            for n in range(N):
                nc.vector.tensor_scalar(
                    out=dm[:, n, :],
                    in0=eye,
                    scalar1=es[:, n : n + 1],
                    scalar2=r,
                    op0=OP.mult,
                    op1=OP.mult,
                )

            # h = sum_n diag(alpha_n) @ x_n, accumulated in PSUM per d-half
            hsb = hpool.tile([P, D], f32)
            for dh in range(D // DH):
                ps = pp.tile([P, DH], f32, tag=f"ps{dh}")
                for n in range(N):
                    lhsT = dm[:, n, :]
                    rhs = x[:, n, dh * DH : (dh + 1) * DH]
                    if use_f32r:
                        lhsT = lhsT.bitcast(mybir.dt.float32r)
                        rhs = rhs.bitcast(mybir.dt.float32r)
                    nc.tensor.matmul(ps, lhsT, rhs, start=(n == 0), stop=(n == N - 1))
                nc.scalar.copy(out=hsb[:, dh * DH : (dh + 1) * DH], in_=ps)
            nc.sync.dma_start(out=h_v[t], in_=hsb)

        nc.sync.dma_start(out=am_v, in_=am_stage)

    nc.finalize()
    return nc


def _get_nc():
    key = (TPC, USE_F32R)
    if key not in _nc_cache:
        _nc_cache[key] = build_nc()
    return _nc_cache[key]


def _host_prep(inputs):
    sources = np.ascontiguousarray(np.asarray(inputs["sources"], dtype=np.float32))
    w_all = np.asarray(inputs["w_all"], dtype=np.float32)
    mem_bias = np.asarray(inputs["mem_bias"], dtype=np.float32)
    recent_bias = np.asarray(inputs["recent_bias"], dtype=np.float32)
    norm_weight = np.asarray(inputs["norm_weight"], dtype=np.float32)
    ridx = int(np.asarray(inputs["router_idx"]))
    wv = np.ascontiguousarray((w_all[ridx] * norm_weight).astype(np.float32))
    bias = np.zeros((N,), dtype=np.float32)
    bias[0] = mem_bias[ridx]
    bias[N - 1] += recent_bias[ridx]
    return sources, wv, bias


def kernel(**inputs):
    return _run(inputs, trace=False)[0]


def kernel_traced(**inputs):
    """Like kernel() but also returns (exec_time_ns, trace_path)."""
    return _run(inputs, trace=True)


def _run(inputs, trace=False):
    from concourse.bass_utils import run_bass_kernel_spmd

    sources, wv, bias = _host_prep(inputs)
    src_flat = sources.reshape(TOK, N * D)
    in_maps = [
        {
            "src": np.ascontiguousarray(src_flat[c * TPC : (c + 1) * TPC]),
            "wv": wv,
            "bias": bias,
        }
        for c in range(NCORES)
    ]
    nc = _get_nc()
    res = run_bass_kernel_spmd(
        nc, in_maps, list(range(NCORES)), trace=trace
    )
    outs = res.results
    h = np.concatenate([outs[c]["h"] for c in range(NCORES)], axis=0)
    am = np.concatenate([outs[c]["am"] for c in range(NCORES)], axis=0)
    h = h.reshape(B, S, D).astype(np.float32)
    am = am.reshape(B, S).astype(np.float32)
    trace_path = None
    if res.instructions_and_trace is not None:
        trace_path = res.instructions_and_trace[1]
    return (h, am), res.exec_time_ns, trace_path
